# revision 2
# baseline (speedup 1.0000x reference)
"""Multi-head causal attention (B=4, T=2048, K=1024, H=16) on 4 NeuronCores.

Sharding: data parallel over B — each core computes the full 16-head causal
attention for one batch. The axon tunnel (~70 MB/s h2d, ~25 MB/s d2h) makes
wire bytes the wall-clock bottleneck, so everything crosses in fp16 and is
minimal: per core x natural-layout (4 MB) plus ONE of the four weight
matrices (2 MB, pre-transposed/chunked on host); an on-device AllGather over
NeuronLink rebuilds the full weight set on every core. The causal mask is
generated on device (affine_select); x is transposed on load by the DMA xbar;
y is transposed on chip (PE transpose) and written in natural [T, K] fp16
layout with the bias already added, so the host does only an fp32 cast.

On-chip: all matmuls in fp16 (fp32 PSUM accumulation). Scores are built
transposed (P~T[u,t] = exp(k.q/4 + bias)) so the PV matmul needs no on-chip
transposes; a ones-column appended to V yields the softmax denominator from
the same matmul; normalization runs in fp32 (1/denom dynamic range exceeds
fp16). The exp bias keeps pt in fp16 range: -6.8 globally (max scaled score
is ~17.1), and -4.5 for the first diagonal window, whose tokens (t<128) draw
all their softmax terms from it — softmax is invariant to a per-row shift, so
mixing biases across windows that serve disjoint token rows is exact. Heads
are processed in pairs occupying PE row groups 0-63 / 64-127; causality skips
all-zero blocks and restricts diagonal blocks to their valid 128-col window.
"""
import sys
sys.path.insert(0, '/opt/trn_rl_repo')
import numpy as np

B, T, K, H = 4, 2048, 1024, 16
S = K // H          # 64 head dim
NF = K // 128       # 8 contraction chunks
NMB = K // 128      # 8 feature blocks (= head pairs)
NTB = T // 512      # 4 t-blocks of 512
NU = T // 128       # 16 u-chunks of 128
SCALE = float(H) ** -0.5  # 0.25
EBIAS = -6.8        # exp bias: keeps exp(score*SCALE+EBIAS) in fp16 range
EBIAS0 = -4.5       # first diagonal window (t<128): smaller scores, and the
                    # lone t=0 term must not underflow fp16 to zero
NCORES = 4

_CACHE = {}


def _build():
    import concourse.tile as tile
    import concourse.mybir as mybir
    from concourse import bacc
    from concourse.masks import make_identity

    dt = mybir.dt
    F16 = dt.float16
    F32 = dt.float32
    F32R = dt.float32r
    AF = mybir.ActivationFunctionType
    MUL = mybir.AluOpType.mult
    ADD = mybir.AluOpType.add

    nc = bacc.Bacc("TRN2", target_bir_lowering=False, debug=False)

    x_d = nc.dram_tensor("x_l", [T, K], F16, kind="ExternalInput")
    # tensor-parallel weight distribution: core c ships only weight matrix c
    # of (Wq, Wk, Wv, Wp); an on-device AllGather over NeuronLink rebuilds the
    # full set on every core (8 MB h2d total instead of 32 MB).
    wsh_d = nc.dram_tensor("wsh", [128, NF, K], F16, kind="ExternalInput")
    wstage_d = nc.dram_tensor("wstage", [128, NF, K], F16, kind="Internal")
    wall_d = nc.dram_tensor("wall", [NCORES, 128, NF, K], F16, kind="Internal")
    bp_d = nc.dram_tensor("bp_l", [128, NMB], F32, kind="ExternalInput")
    y_d = nc.dram_tensor("y", [NTB, 4, 128, K], F16, kind="ExternalOutput")

    with tile.TileContext(nc) as tc:
        nc.sync.dma_start(wstage_d[:], wsh_d[:])
        nc.gpsimd.collective_compute(
            "AllGather",
            mybir.AluOpType.bypass,
            replica_groups=[list(range(NCORES))],
            ins=[wstage_d[:]],
            outs=[wall_d[:]],
        )
        with tc.tile_pool(name="persist", bufs=1) as pp:

            qT = pp.tile([128, NMB, T], F16)         # 32 KB/part
            kT = pp.tile([128, NMB, T], F16)         # 32 KB/part
            v_sb = pp.tile([128, NU, H, S + 1], F16)  # 33.25 KB/part
            ones_r = pp.tile([1, S], F32R)
            ident = pp.tile([128, 128], F16)
            make_identity(nc, ident[:])
            ebias = pp.tile([128, 1], F32)
            nc.vector.memset(ebias[:], EBIAS)
            ebias0 = pp.tile([128, 1], F32)
            nc.vector.memset(ebias0[:], EBIAS0)

            # ---------------- Phase 1: QKV projections ----------------
            with tc.tile_pool(name="wqkv", bufs=1) as wqkv_pool, \
                 tc.tile_pool(name="xs", bufs=2) as xs_pool, \
                 tc.tile_pool(name="ps_acc", bufs=5, space="PSUM") as ps_acc, \
                 tc.tile_pool(name="ps_v", bufs=2, space="PSUM") as ps_v:
                wq = wqkv_pool.tile([128, NF, K], F16, tag="wq")  # 16 KB/part
                wk = wqkv_pool.tile([128, NF, K], F16, tag="wk")
                wv = wqkv_pool.tile([128, NF, K], F16, tag="wv")
                nc.sync.dma_start(wq[:], wall_d[0])
                nc.sync.dma_start(wk[:], wall_d[1])
                nc.sync.dma_start(wv[:], wall_d[2])

                ones_f = wqkv_pool.tile([1, S], F32, tag="ones_f")
                nc.vector.memset(ones_f[:], 1.0)
                nc.vector.tensor_copy(ones_r[:], ones_f[:])
                nc.vector.memset(v_sb[:, :, :, S:S + 1], 1.0)

                for tb in range(NTB):  # 512-token blocks
                    x_tb = xs_pool.tile([128, NF, 512], F16, tag="x")
                    for f in range(NF):  # xbar transpose: [512,128] -> [128,512]
                        nc.sync.dma_start_transpose(
                            x_tb[:, f, :],
                            x_d[tb * 512:(tb + 1) * 512,
                                f * 128:(f + 1) * 128])
                    for w_sb, dst in ((wq, qT), (wk, kT)):
                        for mb in range(NMB):
                            acc = ps_acc.tile([128, 512], F32, tag="acc")
                            for f in range(NF):
                                nc.tensor.matmul(
                                    acc[:],
                                    w_sb[:, f, mb * 128:(mb + 1) * 128],
                                    x_tb[:, f, :],
                                    start=(f == 0), stop=(f == NF - 1),
                                )
                            nc.vector.tensor_copy(
                                dst[:, mb, tb * 512:(tb + 1) * 512], acc[:])
                    for tt in range(4):
                        ub = tb * 4 + tt
                        for vh in range(2):
                            accv = ps_v.tile([128, 512], F32, tag="v")
                            for f in range(NF):
                                nc.tensor.matmul(
                                    accv[:],
                                    x_tb[:, f, tt * 128:(tt + 1) * 128],
                                    wv[:, f, vh * 512:(vh + 1) * 512],
                                    start=(f == 0), stop=(f == NF - 1),
                                )
                            nc.vector.tensor_copy(
                                v_sb[:, ub, vh * 8:(vh + 1) * 8, 0:S],
                                accv[:].rearrange("p (h s) -> p h s", h=8),
                            )

            # -------- Phase 2+3: causal attention + output projection --------
            with tc.tile_pool(name="wp_pool", bufs=1) as wp_pool, \
                 tc.tile_pool(name="outa", bufs=2) as outa_pool, \
                 tc.tile_pool(name="pexp", bufs=8) as pexp, \
                 tc.tile_pool(name="small", bufs=2) as sm, \
                 tc.tile_pool(name="ysb", bufs=2) as ysb_pool, \
                 tc.tile_pool(name="ps_sc", bufs=3, space="PSUM") as ps_sc, \
                 tc.tile_pool(name="ps_pv", bufs=1, space="PSUM") as ps_pv, \
                 tc.tile_pool(name="ps_bc", bufs=1, space="PSUM") as ps_bc, \
                 tc.tile_pool(name="ps_y", bufs=2, space="PSUM") as ps_y:
                wp_sb = wp_pool.tile([128, NMB, K], F16, tag="wp")  # 16 KB/part
                nc.sync.dma_start(wp_sb[:], wall_d[3])
                msk = wp_pool.tile([128, 128], F32, tag="msk")
                # additive causal mask: 0 where u <= t, -60000 where u > t
                nc.gpsimd.memset(msk[:], 0.0)
                nc.gpsimd.affine_select(
                    out=msk[:], in_=msk[:],
                    compare_op=mybir.AluOpType.is_ge,
                    fill=-60000.0, base=0,
                    channel_multiplier=-1, pattern=[[1, 128]])
                bp_sb = wp_pool.tile([128, NMB], F32, tag="bp")
                nc.sync.dma_start(bp_sb[:], bp_d[:])

                for tb in range(NTB):
                    outA = outa_pool.tile([128, NMB, 512], F16, tag="outa")
                    y_nat = ysb_pool.tile([128, 4, K], F16, tag="ynat",
                                          name=f"yn{tb}")
                    nu = 4 * tb + 4
                    for hp in range(NMB):
                        pv0 = ps_pv.tile([S + 1, 512], F32, tag="pv0",
                                         name=f"pv0_{tb}_{hp}")
                        pv1 = ps_pv.tile([S + 1, 512], F32, tag="pv1",
                                         name=f"pv1_{tb}_{hp}")
                        for ub in range(nu):
                            # valid columns: t >= u  =>  t_local >= 128*j
                            j = ub - 4 * tb
                            w0 = 128 * j if j > 0 else 0
                            sc0 = ps_sc.tile([128, 512], F32, tag="sc",
                                             name=f"sc0_{tb}_{hp}_{ub}")
                            sc1 = ps_sc.tile([128, 512], F32, tag="sc",
                                             name=f"sc1_{tb}_{hp}_{ub}")
                            # paired score matmuls: PE row groups 0-63 / 64-127
                            nc.tensor.matmul(
                                sc0[:, w0:512],
                                kT[0:S, hp, ub * 128:(ub + 1) * 128],
                                qT[0:S, hp, tb * 512 + w0:(tb + 1) * 512],
                                start=True, stop=True,
                            )
                            nc.tensor.matmul(
                                sc1[:, w0:512],
                                kT[S:128, hp, ub * 128:(ub + 1) * 128],
                                qT[S:128, hp, tb * 512 + w0:(tb + 1) * 512],
                                start=True, stop=True,
                            )
                            if j >= 0:  # diagonal: additive causal mask on
                                # the fp32 scores (pre-exp: fp16 exp of a
                                # masked-but-large score would be inf)
                                nc.vector.tensor_tensor(
                                    sc0[:, w0:w0 + 128], sc0[:, w0:w0 + 128],
                                    msk[:], ADD)
                                nc.vector.tensor_tensor(
                                    sc1[:, w0:w0 + 128], sc1[:, w0:w0 + 128],
                                    msk[:], ADD)
                            pt0 = pexp.tile([128, 512], F16, tag="pt",
                                            name=f"pt0_{tb}_{hp}_{ub}")
                            pt1 = pexp.tile([128, 512], F16, tag="pt",
                                            name=f"pt1_{tb}_{hp}_{ub}")
                            if tb == 0 and ub == 0:
                                # tokens t<128 take all their softmax terms
                                # from columns [0:128] of this block: a
                                # different bias there is self-consistent
                                for pt, sc in ((pt0, sc0), (pt1, sc1)):
                                    nc.scalar.activation(
                                        pt[:, 0:128], sc[:, 0:128],
                                        AF.Exp, bias=ebias0[:], scale=SCALE)
                                    nc.scalar.activation(
                                        pt[:, 128:512], sc[:, 128:512],
                                        AF.Exp, bias=ebias[:], scale=SCALE)
                            else:
                                nc.scalar.activation(
                                    pt0[:, w0:512], sc0[:, w0:512],
                                    AF.Exp, bias=ebias[:], scale=SCALE)
                                nc.scalar.activation(
                                    pt1[:, w0:512], sc1[:, w0:512],
                                    AF.Exp, bias=ebias[:], scale=SCALE)
                            nc.tensor.matmul(
                                pv0[:, w0:512], v_sb[:, ub, 2 * hp, :],
                                pt0[:, w0:512],
                                start=(ub == 0), stop=(ub == nu - 1),
                            )
                            nc.tensor.matmul(
                                pv1[:, w0:512], v_sb[:, ub, 2 * hp + 1, :],
                                pt1[:, w0:512],
                                start=(ub == 0), stop=(ub == nu - 1),
                            )
                        # normalize: out[s,t] = pv[s,t] / pv[S,t]  (fp32 path:
                        # 1/denom spans ~1e12 dynamic range, too wide for fp16)
                        for pv, po in ((pv0, 0), (pv1, S)):
                            recip = sm.tile([1, 512], F32, tag="recip",
                                            name=f"rc_{tb}_{hp}_{po}")
                            nc.vector.reciprocal(recip[:], pv[S:S + 1, :])
                            recip_r = sm.tile([1, 512], F32R, tag="recip_r",
                                              name=f"rr_{tb}_{hp}_{po}")
                            nc.vector.tensor_copy(recip_r[:], recip[:])
                            bc = ps_bc.tile([S, 512], F32, tag="bc",
                                            name=f"bc_{tb}_{hp}_{po}")
                            nc.tensor.matmul(bc[:], ones_r[:], recip_r[:],
                                             start=True, stop=True)
                            bc_sb = sm.tile([S, 512], F32, tag="bc_sb",
                                            name=f"bs_{tb}_{hp}_{po}")
                            nc.vector.tensor_copy(bc_sb[:], bc[:])
                            nc.vector.tensor_tensor(
                                outA[po:po + S, hp, :], pv[0:S, :],
                                bc_sb[:], MUL)

                    # ---- output projection + bias for this t-block ----
                    for jb in range(K // 128):
                        yt = ps_y.tile([128, 512], F32, tag="y",
                                       name=f"yt{tb}_{jb}")
                        for i in range(NMB):
                            nc.tensor.matmul(
                                yt[:],
                                wp_sb[:, i, jb * 128:(jb + 1) * 128],
                                outA[:, i, :],
                                start=(i == 0), stop=(i == NMB - 1),
                            )
                        ysb = ysb_pool.tile([128, 512], F16, tag="ysb",
                                            name=f"ys{tb}_{jb}")
                        nc.vector.tensor_scalar(
                            ysb[:], yt[:], bp_sb[:, jb:jb + 1], None, ADD)
                        # transpose [j,t] -> [t,j] so the output DMA writes
                        # natural [T, K] layout (no host-side transpose)
                        ytr = ps_sc.tile([128, 512], F16, tag="sc",
                                         name=f"ytr{tb}_{jb}")
                        for tt in range(4):
                            nc.tensor.transpose(
                                ytr[:, tt * 128:(tt + 1) * 128],
                                ysb[:, tt * 128:(tt + 1) * 128],
                                ident[:])
                        nc.vector.tensor_copy(
                            y_nat[:, :, jb * 128:(jb + 1) * 128],
                            ytr[:].rearrange("p (a b) -> p a b", a=4))
                    for tt in range(4):
                        nc.sync.dma_start(y_d[tb, tt], y_nat[:, tt, :])

    nc.compile()
    return nc


def _prep_inputs(input_data, Wq, Wk, Wv, Wp, bp):
    """Build the 4 per-core input maps (fp16 wire format, host-side layout)."""
    f16 = np.float16

    def chunkT(W):  # [m, k] natural -> [128, NF, m] chunked-transposed
        return np.ascontiguousarray(
            W.astype(f16).reshape(W.shape[0], NF, 128).transpose(2, 1, 0))

    wsh = [chunkT(Wq), chunkT(Wk), chunkT(Wv), chunkT(Wp)]
    bp_l = np.ascontiguousarray(bp.astype(np.float32).reshape(NMB, 128).T)

    in_maps = []
    for b in range(NCORES):
        in_maps.append({
            "x_l": input_data[b].astype(f16), "wsh": wsh[b], "bp_l": bp_l,
        })
    return in_maps


def _enable_jax_compile_cache():
    try:
        import jax
        jax.config.update("jax_compilation_cache_dir", "/tmp/jax_kernel_cache")
        jax.config.update("jax_persistent_cache_min_entry_size_bytes", -1)
        jax.config.update("jax_persistent_cache_min_compile_time_secs", 0.0)
    except Exception:
        pass


def kernel(input_data, Wq, Wk, Wv, Wp, bp, _trace=False):
    from concourse.bass_utils import run_bass_kernel_spmd

    _enable_jax_compile_cache()
    if "nc" not in _CACHE:
        _CACHE["nc"] = _build()
    nc = _CACHE["nc"]

    in_maps = _prep_inputs(
        np.asarray(input_data), np.asarray(Wq), np.asarray(Wk),
        np.asarray(Wv), np.asarray(Wp), np.asarray(bp))

    br = run_bass_kernel_spmd(nc, in_maps, core_ids=list(range(NCORES)),
                              trace=_trace)
    _CACHE["last_result"] = br

    y = np.empty((B, T, K), np.float32)
    for b in range(B):
        y[b] = br.results[b]["y"].reshape(T, K)  # fp16 -> fp32 cast
    return y
